# revision 26
# baseline (speedup 1.0000x reference)
"""Trainium2 Bass kernel for a single DeBERTa-style attention head.

Problem shapes (hardcoded):
  B=8, S=2048, E=768(n_embed), H=64(head)
  q = I @ Wq + bq ; k = x @ Wk + bk ; v = x @ Wv + bv
  w = (q @ k^T) / sqrt(E) ; w = where(mask==0, -1e9, w)
  scores = softmax(w, axis=-1) ; out = scores @ v

Sharding: data-parallel over batch B across the 8 NeuronCores (one batch
element per core, identical SPMD program). Host-side (inside kernel()) the
per-core slices are laid out transposed (I^T, x^T, mask^T) so the device
never has to transpose bulk data: PE transposes cost ~300ns per 128x128
block and suppress the PE clock-gate warmup, which dominated the v1 profile.

Per-core dataflow (bf16 operands, fp32 PSUM accumulation):
  1. Cast-DMA (SWDGE fp32->bf16) I^T, x^T into SBUF with embed on partitions.
  2. qT,kT [64,2048] = Wq/Wk-chunk (stationary) x I^T/x^T (streaming) + rank-1
     bias matmul; v per k-chunk + bias + a ones column (v_aug) so the softmax
     denominator falls out of the second matmul's extra output column.
  3. k-chunk-major attention: w^T-chunk [128k, q] = kT-chunk^T @ qT;
     e = exp(w^T * 1/sqrt(E)) on ACT straight from PSUM (no row max needed:
     |w/sqrt(E)| is O(1) so exp cannot overflow, and softmax is
     shift-invariant); s^T = e * mask^T (mask int32 DMA-cast to bf16;
     multiplicative masking matches the reference's -1e9 additive mask, which
     underflows to exactly 0 after softmax); ctx[q-chunk, 0:65] accumulates
     s^T-chunk^T @ v_aug over all 16 k-chunks in PSUM.
  4. out = ctx[:,0:64] * (1/ctx[:,64]).
"""

import math
from contextlib import ExitStack

import numpy as np

import concourse.bass as bass
import concourse.tile as tile
import concourse.mybir as mybir
from concourse import bacc
from concourse.bass_utils import run_bass_kernel_spmd

B, S, E, H = 8, 2048, 768, 64
N_CORES = 8
SC = S // 128   # 16 seq chunks
EC = E // 128   # 6 embed chunks
SCALE = 1.0 / math.sqrt(E)

F32 = mybir.dt.float32
BF16 = mybir.dt.bfloat16
I32 = mybir.dt.int32
AF = mybir.ActivationFunctionType
ALU = mybir.AluOpType

_cache = {}


def _build_program():
    nc = bacc.Bacc("TRN2", target_bir_lowering=False, debug=False)

    # Host feeds these already transposed: IT/XT are [E, S], maskT is [S, S]
    # with [k, q] indexing. x rides HWDGE as float32r (runs concurrently with
    # the SWDGE stream, so the k/v path starts ~20us earlier and the SWDGE
    # stream is 6MB shorter); I rides the SWDGE cast path.
    F32R = mybir.dt.float32r
    dIT = nc.dram_tensor("IT", [E, S], F32, kind="ExternalInput")
    dXT = nc.dram_tensor("XT", [E, S], F32R, kind="ExternalInput")
    dmT = nc.dram_tensor("maskT", [S, S], I32, kind="ExternalInput")
    dWq = nc.dram_tensor("Wq", [E, H], BF16, kind="ExternalInput")
    dWk = nc.dram_tensor("Wk", [E, H], F32R, kind="ExternalInput")
    dWv = nc.dram_tensor("Wv", [E, H], F32R, kind="ExternalInput")
    # bq/bk arrive host-broadcast as [H, H] so the DMA moves contiguous
    # 128B-per-partition rows instead of 2B fragments; only col 0 is used
    # (per-partition bias on the h axis of qT/kT)
    dbq = nc.dram_tensor("bq", [H, H], F32, kind="ExternalInput")
    dbk = nc.dram_tensor("bk", [H, H], F32, kind="ExternalInput")
    dbv = nc.dram_tensor("bv", [1, H], BF16, kind="ExternalInput")
    dout = nc.dram_tensor("out", [S, H], F32, kind="ExternalOutput")

    with tile.TileContext(nc) as tc, ExitStack() as ctx:
        singles = ctx.enter_context(tc.tile_pool(name="singles", bufs=1))

        # big input loads go first on each DGE queue
        IT = singles.tile([128, EC, S], BF16, tag="IT")
        XT = singles.tile([128, EC, S], F32R, tag="XT")
        nc.gpsimd.dma_start(
            out=IT, in_=dIT.ap().rearrange("(ec p) s -> p ec s", p=128)
        )
        nc.sync.dma_start(
            out=XT, in_=dXT.ap().rearrange("(ec p) s -> p ec s", p=128)
        )

        ones_row = singles.tile([1, S], BF16, tag="ones")
        nc.vector.memset(ones_row, 1.0)

        w_sb = {}
        for name, dram, dt_ in (
            ("Wk", dWk, F32R),
            ("Wv", dWv, F32R),
            ("Wq", dWq, BF16),
        ):
            t = singles.tile([128, EC, H], dt_, tag=name)
            nc.sync.dma_start(
                out=t, in_=dram.ap().rearrange("(ec p) h -> p ec h", p=128)
            )
            w_sb[name] = t
        b_sb = {}
        for name, dram, shape, dt_ in (
            ("bq", dbq, [H, H], F32),
            ("bk", dbk, [H, H], F32),
            ("bv", dbv, [1, H], BF16),
        ):
            t = singles.tile(shape, dt_, tag=name)
            nc.sync.dma_start(out=t, in_=dram.ap())
            b_sb[name] = t

        # ---- projections ----
        qT = singles.tile([64, S], BF16, tag="qT")
        kT = singles.tile([64, S], BF16, tag="kT")
        vA = singles.tile([128, SC, 66], BF16, tag="vA")

        # whole mask^T resident in SBUF (64KB/partition), filled by 2MB-read
        # SWDGE cast-DMAs issued back-to-back so the SDMA stream never gates
        # on consumer pool slots (SWDGE-cast sustains ~330GB/s; HWDGE bulk
        # measured slower, and the combined rate is HBM-capped anyway)
        maskT_all = singles.tile([128, SC, S], BF16, tag="maskT")
        for ki in range(0, SC, 2):
            nc.gpsimd.dma_start(
                out=maskT_all[:, ki:ki + 2, :],
                in_=dmT.ap()[ki * 128:(ki + 2) * 128, :].rearrange(
                    "(t p) q -> p t q", p=128
                ),
            )

        psw = ctx.enter_context(tc.tile_pool(name="psw", bufs=2, space="PSUM"))
        ep = ctx.enter_context(tc.tile_pool(name="ep", bufs=3))
        outp = ctx.enter_context(tc.tile_pool(name="outp", bufs=3))

        def emit_score(ki):
            """w^T-chunk -> exp -> mask multiply; returns the sT tile."""
            maskT_sb = maskT_all[:, ki, :]
            sT_sb = ep.tile([128, S], BF16, tag="sT")
            for hh in range(2):
                wp = psw.tile([128, 1024], F32, tag="w")
                for nb in range(2):
                    nc.tensor.matmul(
                        wp[:, nb * 512:(nb + 1) * 512],
                        lhsT=kT[:, ki * 128:(ki + 1) * 128],
                        rhs=qT[:, (hh * 2 + nb) * 512:(hh * 2 + nb + 1) * 512],
                        start=True,
                        stop=True,
                    )
                e_sb = ep.tile([128, 1024], BF16, tag="e")
                nc.scalar.activation(e_sb, wp, AF.Exp, scale=SCALE)
                nc.vector.tensor_tensor(
                    sT_sb[:, hh * 1024:(hh + 1) * 1024],
                    e_sb,
                    maskT_sb[:, hh * 1024:(hh + 1) * 1024],
                    ALU.mult,
                )
            return sT_sb

        with tc.tile_pool(name="ps2", bufs=2, space="PSUM") as ps2:
            for srcT, wname, bname, dstT in (
                (XT, "Wk", "bk", kT),
                (IT, "Wq", "bq", qT),
            ):
                for nb in range(4):
                    ps = ps2.tile([64, 512], F32, tag="pqk")
                    for ei in range(EC):
                        nc.tensor.matmul(
                            ps,
                            lhsT=w_sb[wname][:, ei, :],
                            rhs=srcT[:, ei, nb * 512:(nb + 1) * 512],
                            start=(ei == 0),
                            stop=(ei == EC - 1),
                        )
                    # bias folded into the PSUM->SBUF copy on DVE
                    # (per-partition scalar add keeps ACT free for the exps)
                    nc.vector.tensor_scalar(
                        dstT[:, nb * 512:(nb + 1) * 512],
                        ps,
                        b_sb[bname][:, 0:1],
                        None,
                        ALU.add,
                    )

            # ki=0 scores go first so the exp/mask chain starts as early as
            # possible; the v projections then fill PE slack behind it.
            sT0 = emit_score(0)

            for kb in range(SC):
                psv = ps2.tile([128, H], F32, tag="pv")
                for ei in range(EC):
                    nc.tensor.matmul(
                        psv,
                        lhsT=XT[:, ei, kb * 128:(kb + 1) * 128],
                        rhs=w_sb["Wv"][:, ei, :],
                        start=(ei == 0),
                        stop=False,
                    )
                nc.tensor.matmul(
                    psv,
                    lhsT=ones_row[:, 0:128],
                    rhs=b_sb["bv"],
                    start=False,
                    stop=True,
                )
                nc.vector.tensor_copy(vA[:, kb, 0:H], psv)
                nc.vector.memset(vA[:, kb, H:H + 1], 1.0)

        # ---- attention, k-chunk-major ----
        psctx = ctx.enter_context(tc.tile_pool(name="psctx", bufs=1, space="PSUM"))

        # [q_within, qj, 64 ctx + 1 denom + pad] — 128-wide regions keep each
        # accumulation group inside one PSUM bank.
        ctxall = psctx.tile([128, SC, 128], F32, tag="ctxall")

        def emit_ctx(ki, sT_sb):
            # start=True zeroes the whole 2KB PSUM bank, so only the first
            # matmul touching each bank (4 qj regions per bank) gets it; the
            # other ki=0 writes land on zeroed-has_written elements and
            # overwrite. stop on the bank's last matmul.
            for qj in range(SC):
                nc.tensor.matmul(
                    ctxall[:, qj, 0:H + 1],
                    lhsT=sT_sb[:, qj * 128:(qj + 1) * 128],
                    rhs=vA[:, ki, 0:H + 1],
                    start=(ki == 0 and qj % 4 == 0),
                    stop=(ki == SC - 1 and qj % 4 == 3),
                )

        # Software-pipelined by one k-chunk: ctx matmuls for ki-1 are emitted
        # after the w matmuls of ki, so the PE never waits on the exp/mask
        # chain of the chunk it just computed (keeps the PE dense and the HAM
        # clock gate warm).
        prev = (0, sT0)
        for ki in range(1, SC):
            sT_sb = emit_score(ki)
            emit_ctx(prev[0], prev[1])
            prev = (ki, sT_sb)
        emit_ctx(prev[0], prev[1])

        # vectorized epilogue: one reciprocal over all 16 denominators, one
        # free-dim-broadcast multiply, one 512KB output DMA
        recip_t = outp.tile([128, SC, 1], F32, tag="recip")
        nc.vector.reciprocal(recip_t, ctxall[:, :, H:H + 1])
        recip_bcast = bass.AP(
            tensor=recip_t.tensor,
            offset=recip_t.offset,
            ap=[recip_t.ap[0], recip_t.ap[1], [0, H]],
        )
        o_all = outp.tile([128, SC, H], F32, tag="o")
        nc.vector.tensor_tensor(o_all, ctxall[:, :, 0:H], recip_bcast, ALU.mult)
        nc.sync.dma_start(
            out=dout.ap().rearrange("(qj p) h -> p qj h", p=128), in_=o_all
        )

    nc.compile()
    return nc


def get_program():
    if "nc" not in _cache:
        _cache["nc"] = _build_program()
    return _cache["nc"]


def make_in_maps(I, x, mask, Wq, bq, Wk, bk, Wv, bv):
    I = np.asarray(I, dtype=np.float32)
    x = np.asarray(x, dtype=np.float32)
    mask = np.asarray(mask, dtype=np.int32)
    import ml_dtypes

    BF = ml_dtypes.bfloat16
    Wq = np.asarray(Wq, dtype=np.float32).astype(BF)
    Wk = np.asarray(Wk, dtype=np.float32)
    Wv = np.asarray(Wv, dtype=np.float32)
    bq = np.broadcast_to(
        np.asarray(bq, dtype=np.float32).reshape(H, 1), (H, H)
    ).copy()
    bk = np.broadcast_to(
        np.asarray(bk, dtype=np.float32).reshape(H, 1), (H, H)
    ).copy()
    bv = np.asarray(bv, dtype=np.float32).reshape(1, H).astype(BF)

    return [
        {
            "IT": np.ascontiguousarray(I[b].T),
            "XT": np.ascontiguousarray(x[b].T),
            "maskT": np.ascontiguousarray(mask[b].T),
            "Wq": Wq, "Wk": Wk, "Wv": Wv,
            "bq": bq, "bk": bk, "bv": bv,
        }
        for b in range(B)
    ]


def kernel(I, x, mask, Wq, bq, Wk, bk, Wv, bv):
    nc = get_program()
    in_maps = make_in_maps(I, x, mask, Wq, bq, Wk, bk, Wv, bv)
    res = run_bass_kernel_spmd(nc, in_maps, list(range(N_CORES)))
    out = np.stack([res.results[b]["out"] for b in range(B)], axis=0)
    return out.astype(np.float32)


# revision 27
# speedup vs baseline: 1.3808x; 1.3808x over previous
"""Trainium2 Bass kernel for a single DeBERTa-style attention head.

Problem shapes (hardcoded):
  B=8, S=2048, E=768(n_embed), H=64(head)
  q = I @ Wq + bq ; k = x @ Wk + bk ; v = x @ Wv + bv
  w = (q @ k^T) / sqrt(E) ; w = where(mask==0, -1e9, w)
  scores = softmax(w, axis=-1) ; out = scores @ v

Sharding: data-parallel over batch B across the 8 NeuronCores (one batch
element per core, identical SPMD program). Host-side (inside kernel()) the
per-core slices are laid out transposed (I^T, x^T, mask^T) so the device
never has to transpose bulk data: PE transposes cost ~300ns per 128x128
block and suppress the PE clock-gate warmup, which dominated the v1 profile.

Per-core dataflow (bf16 operands, fp32 PSUM accumulation):
  1. Cast-DMA (SWDGE fp32->bf16) I^T, x^T into SBUF with embed on partitions.
  2. qT,kT [64,2048] = Wq/Wk-chunk (stationary) x I^T/x^T (streaming) + rank-1
     bias matmul; v per k-chunk + bias + a ones column (v_aug) so the softmax
     denominator falls out of the second matmul's extra output column.
  3. k-chunk-major attention: w^T-chunk [128k, q] = kT-chunk^T @ qT;
     e = exp(w^T * 1/sqrt(E)) on ACT straight from PSUM (no row max needed:
     |w/sqrt(E)| is O(1) so exp cannot overflow, and softmax is
     shift-invariant); s^T = e * mask^T (mask int32 DMA-cast to bf16;
     multiplicative masking matches the reference's -1e9 additive mask, which
     underflows to exactly 0 after softmax); ctx[q-chunk, 0:65] accumulates
     s^T-chunk^T @ v_aug over all 16 k-chunks in PSUM.
  4. out = ctx[:,0:64] * (1/ctx[:,64]).
"""

import math
from contextlib import ExitStack

import numpy as np

import concourse.bass as bass
import concourse.tile as tile
import concourse.mybir as mybir
from concourse import bacc
from concourse.bass_utils import run_bass_kernel_spmd

B, S, E, H = 8, 2048, 768, 64
N_CORES = 8
SC = S // 128   # 16 seq chunks
EC = E // 128   # 6 embed chunks
SCALE = 1.0 / math.sqrt(E)

F32 = mybir.dt.float32
BF16 = mybir.dt.bfloat16
I32 = mybir.dt.int32
AF = mybir.ActivationFunctionType
ALU = mybir.AluOpType

_cache = {}


def _build_program():
    nc = bacc.Bacc("TRN2", target_bir_lowering=False, debug=False)

    # Host feeds these already transposed: IT/XT are [E, S], maskT is [S, S]
    # with [k, q] indexing. x rides HWDGE as float32r (runs concurrently with
    # the SWDGE stream, so the k/v path starts ~20us earlier and the SWDGE
    # stream is 6MB shorter); I rides the SWDGE cast path.
    F32R = mybir.dt.float32r
    dIT = nc.dram_tensor("IT", [E, S], F32, kind="ExternalInput")
    dXT = nc.dram_tensor("XT", [E, S], F32, kind="ExternalInput")
    dmT = nc.dram_tensor("maskT", [S, S], I32, kind="ExternalInput")
    dWq = nc.dram_tensor("Wq", [E, H], BF16, kind="ExternalInput")
    dWk = nc.dram_tensor("Wk", [E, H], BF16, kind="ExternalInput")
    dWv = nc.dram_tensor("Wv", [E, H], BF16, kind="ExternalInput")
    # bq/bk arrive host-broadcast as [H, H] so the DMA moves contiguous
    # 128B-per-partition rows instead of 2B fragments; only col 0 is used
    # (per-partition bias on the h axis of qT/kT)
    dbq = nc.dram_tensor("bq", [H, H], F32, kind="ExternalInput")
    dbk = nc.dram_tensor("bk", [H, H], F32, kind="ExternalInput")
    dbv = nc.dram_tensor("bv", [1, H], BF16, kind="ExternalInput")
    dout = nc.dram_tensor("out", [S, H], F32, kind="ExternalOutput")

    with tile.TileContext(nc) as tc, ExitStack() as ctx:
        singles = ctx.enter_context(tc.tile_pool(name="singles", bufs=1))

        # big input loads go first on each DGE queue
        IT = singles.tile([128, EC, S], BF16, tag="IT")
        XT = singles.tile([128, EC, S], BF16, tag="XT")
        nc.gpsimd.dma_start(
            out=IT, in_=dIT.ap().rearrange("(ec p) s -> p ec s", p=128)
        )
        nc.gpsimd.dma_start(
            out=XT, in_=dXT.ap().rearrange("(ec p) s -> p ec s", p=128)
        )

        ones_row = singles.tile([1, S], BF16, tag="ones")
        nc.vector.memset(ones_row, 1.0)

        w_sb = {}
        for name, dram, dt_ in (
            ("Wq", dWq, BF16),
            ("Wk", dWk, BF16),
            ("Wv", dWv, BF16),
        ):
            t = singles.tile([128, EC, H], dt_, tag=name)
            nc.sync.dma_start(
                out=t, in_=dram.ap().rearrange("(ec p) h -> p ec h", p=128)
            )
            w_sb[name] = t
        b_sb = {}
        for name, dram, shape, dt_ in (
            ("bq", dbq, [H, H], F32),
            ("bk", dbk, [H, H], F32),
            ("bv", dbv, [1, H], BF16),
        ):
            t = singles.tile(shape, dt_, tag=name)
            nc.sync.dma_start(out=t, in_=dram.ap())
            b_sb[name] = t

        # ---- projections ----
        qT = singles.tile([64, S], BF16, tag="qT")
        kT = singles.tile([64, S], BF16, tag="kT")
        vA = singles.tile([128, SC, 66], BF16, tag="vA")

        # whole mask^T resident in SBUF (64KB/partition), filled by 2MB-read
        # SWDGE cast-DMAs issued back-to-back so the SDMA stream never gates
        # on consumer pool slots (SWDGE-cast sustains ~330GB/s; HWDGE bulk
        # measured slower, and the combined rate is HBM-capped anyway)
        maskT_all = singles.tile([128, SC, S], BF16, tag="maskT")
        for ki in range(0, SC, 2):
            nc.gpsimd.dma_start(
                out=maskT_all[:, ki:ki + 2, :],
                in_=dmT.ap()[ki * 128:(ki + 2) * 128, :].rearrange(
                    "(t p) q -> p t q", p=128
                ),
            )

        psw = ctx.enter_context(tc.tile_pool(name="psw", bufs=2, space="PSUM"))
        ep = ctx.enter_context(tc.tile_pool(name="ep", bufs=3))
        outp = ctx.enter_context(tc.tile_pool(name="outp", bufs=3))

        def emit_score(ki):
            """w^T-chunk -> exp -> mask multiply; returns the sT tile."""
            maskT_sb = maskT_all[:, ki, :]
            sT_sb = ep.tile([128, S], BF16, tag="sT")
            for hh in range(2):
                wp = psw.tile([128, 1024], F32, tag="w")
                for nb in range(2):
                    nc.tensor.matmul(
                        wp[:, nb * 512:(nb + 1) * 512],
                        lhsT=kT[:, ki * 128:(ki + 1) * 128],
                        rhs=qT[:, (hh * 2 + nb) * 512:(hh * 2 + nb + 1) * 512],
                        start=True,
                        stop=True,
                    )
                e_sb = ep.tile([128, 1024], BF16, tag="e")
                nc.scalar.activation(e_sb, wp, AF.Exp, scale=SCALE)
                nc.vector.tensor_tensor(
                    sT_sb[:, hh * 1024:(hh + 1) * 1024],
                    e_sb,
                    maskT_sb[:, hh * 1024:(hh + 1) * 1024],
                    ALU.mult,
                )
            return sT_sb

        with tc.tile_pool(name="ps2", bufs=2, space="PSUM") as ps2:
            for srcT, wname, bname, dstT in (
                (IT, "Wq", "bq", qT),
                (XT, "Wk", "bk", kT),
            ):
                for nb in range(4):
                    ps = ps2.tile([64, 512], F32, tag="pqk")
                    for ei in range(EC):
                        nc.tensor.matmul(
                            ps,
                            lhsT=w_sb[wname][:, ei, :],
                            rhs=srcT[:, ei, nb * 512:(nb + 1) * 512],
                            start=(ei == 0),
                            stop=(ei == EC - 1),
                        )
                    # bias folded into the PSUM->SBUF copy on DVE
                    # (per-partition scalar add keeps ACT free for the exps)
                    nc.vector.tensor_scalar(
                        dstT[:, nb * 512:(nb + 1) * 512],
                        ps,
                        b_sb[bname][:, 0:1],
                        None,
                        ALU.add,
                    )

            # ki=0 scores go first so the exp/mask chain starts as early as
            # possible; the v projections then fill PE slack behind it.
            sT0 = emit_score(0)

            for kb in range(SC):
                psv = ps2.tile([128, H], F32, tag="pv")
                for ei in range(EC):
                    nc.tensor.matmul(
                        psv,
                        lhsT=XT[:, ei, kb * 128:(kb + 1) * 128],
                        rhs=w_sb["Wv"][:, ei, :],
                        start=(ei == 0),
                        stop=False,
                    )
                nc.tensor.matmul(
                    psv,
                    lhsT=ones_row[:, 0:128],
                    rhs=b_sb["bv"],
                    start=False,
                    stop=True,
                )
                nc.vector.tensor_copy(vA[:, kb, 0:H], psv)
                nc.vector.memset(vA[:, kb, H:H + 1], 1.0)

        # ---- attention, k-chunk-major ----
        psctx = ctx.enter_context(tc.tile_pool(name="psctx", bufs=1, space="PSUM"))

        # [q_within, qj, 64 ctx + 1 denom + pad] — 128-wide regions keep each
        # accumulation group inside one PSUM bank.
        ctxall = psctx.tile([128, SC, 128], F32, tag="ctxall")

        def emit_ctx(ki, sT_sb):
            # start=True zeroes the whole 2KB PSUM bank, so only the first
            # matmul touching each bank (4 qj regions per bank) gets it; the
            # other ki=0 writes land on zeroed-has_written elements and
            # overwrite. stop on the bank's last matmul.
            for qj in range(SC):
                nc.tensor.matmul(
                    ctxall[:, qj, 0:H + 1],
                    lhsT=sT_sb[:, qj * 128:(qj + 1) * 128],
                    rhs=vA[:, ki, 0:H + 1],
                    start=(ki == 0 and qj % 4 == 0),
                    stop=(ki == SC - 1 and qj % 4 == 3),
                )

        # Software-pipelined by one k-chunk: ctx matmuls for ki-1 are emitted
        # after the w matmuls of ki, so the PE never waits on the exp/mask
        # chain of the chunk it just computed (keeps the PE dense and the HAM
        # clock gate warm).
        prev = (0, sT0)
        for ki in range(1, SC):
            sT_sb = emit_score(ki)
            emit_ctx(prev[0], prev[1])
            prev = (ki, sT_sb)
        emit_ctx(prev[0], prev[1])

        # vectorized epilogue: one reciprocal over all 16 denominators, one
        # free-dim-broadcast multiply, one 512KB output DMA
        recip_t = outp.tile([128, SC, 1], F32, tag="recip")
        nc.vector.reciprocal(recip_t, ctxall[:, :, H:H + 1])
        recip_bcast = bass.AP(
            tensor=recip_t.tensor,
            offset=recip_t.offset,
            ap=[recip_t.ap[0], recip_t.ap[1], [0, H]],
        )
        o_all = outp.tile([128, SC, H], F32, tag="o")
        nc.vector.tensor_tensor(o_all, ctxall[:, :, 0:H], recip_bcast, ALU.mult)
        nc.sync.dma_start(
            out=dout.ap().rearrange("(qj p) h -> p qj h", p=128), in_=o_all
        )

    nc.compile()
    return nc


def get_program():
    if "nc" not in _cache:
        _cache["nc"] = _build_program()
    return _cache["nc"]


def make_in_maps(I, x, mask, Wq, bq, Wk, bk, Wv, bv):
    I = np.asarray(I, dtype=np.float32)
    x = np.asarray(x, dtype=np.float32)
    mask = np.asarray(mask, dtype=np.int32)
    import ml_dtypes

    BF = ml_dtypes.bfloat16
    Wq = np.asarray(Wq, dtype=np.float32).astype(BF)
    Wk = np.asarray(Wk, dtype=np.float32).astype(BF)
    Wv = np.asarray(Wv, dtype=np.float32).astype(BF)
    bq = np.broadcast_to(
        np.asarray(bq, dtype=np.float32).reshape(H, 1), (H, H)
    ).copy()
    bk = np.broadcast_to(
        np.asarray(bk, dtype=np.float32).reshape(H, 1), (H, H)
    ).copy()
    bv = np.asarray(bv, dtype=np.float32).reshape(1, H).astype(BF)

    return [
        {
            "IT": np.ascontiguousarray(I[b].T),
            "XT": np.ascontiguousarray(x[b].T),
            "maskT": np.ascontiguousarray(mask[b].T),
            "Wq": Wq, "Wk": Wk, "Wv": Wv,
            "bq": bq, "bk": bk, "bv": bv,
        }
        for b in range(B)
    ]


def kernel(I, x, mask, Wq, bq, Wk, bk, Wv, bv):
    nc = get_program()
    in_maps = make_in_maps(I, x, mask, Wq, bq, Wk, bk, Wv, bv)
    res = run_bass_kernel_spmd(nc, in_maps, list(range(N_CORES)))
    out = np.stack([res.results[b]["out"] for b in range(B)], axis=0)
    return out.astype(np.float32)


# revision 29
# speedup vs baseline: 1.6298x; 1.1803x over previous
"""Trainium2 Bass kernel for a single DeBERTa-style attention head.

Problem shapes (hardcoded):
  B=8, S=2048, E=768(n_embed), H=64(head)
  q = I @ Wq + bq ; k = x @ Wk + bk ; v = x @ Wv + bv
  w = (q @ k^T) / sqrt(E) ; w = where(mask==0, -1e9, w)
  scores = softmax(w, axis=-1) ; out = scores @ v

Sharding: data-parallel over batch B across the 8 NeuronCores (one batch
element per core, identical SPMD program). Host-side (inside kernel()) the
per-core slices are laid out transposed (I^T, x^T, mask^T) so the device
never has to transpose bulk data: PE transposes cost ~300ns per 128x128
block and suppress the PE clock-gate warmup, which dominated the v1 profile.

Per-core dataflow (bf16 operands, fp32 PSUM accumulation):
  1. Cast-DMA (SWDGE fp32->bf16) I^T, x^T into SBUF with embed on partitions.
  2. qT,kT [64,2048] = Wq/Wk-chunk (stationary) x I^T/x^T (streaming) + rank-1
     bias matmul; v per k-chunk + bias + a ones column (v_aug) so the softmax
     denominator falls out of the second matmul's extra output column.
  3. k-chunk-major attention: w^T-chunk [128k, q] = kT-chunk^T @ qT;
     e = exp(w^T * 1/sqrt(E)) on ACT straight from PSUM (no row max needed:
     |w/sqrt(E)| is O(1) so exp cannot overflow, and softmax is
     shift-invariant); s^T = e * mask^T (mask int32 DMA-cast to bf16;
     multiplicative masking matches the reference's -1e9 additive mask, which
     underflows to exactly 0 after softmax); ctx[q-chunk, 0:65] accumulates
     s^T-chunk^T @ v_aug over all 16 k-chunks in PSUM.
  4. out = ctx[:,0:64] * (1/ctx[:,64]).
"""

import math
from contextlib import ExitStack

import numpy as np

import concourse.bass as bass
import concourse.tile as tile
import concourse.mybir as mybir
from concourse import bacc
from concourse.bass_utils import run_bass_kernel_spmd

B, S, E, H = 8, 2048, 768, 64
N_CORES = 8
SC = S // 128   # 16 seq chunks
EC = E // 128   # 6 embed chunks
SCALE = 1.0 / math.sqrt(E)

F32 = mybir.dt.float32
BF16 = mybir.dt.bfloat16
I32 = mybir.dt.int32
AF = mybir.ActivationFunctionType
ALU = mybir.AluOpType

_cache = {}


def _build_program():
    nc = bacc.Bacc("TRN2", target_bir_lowering=False, debug=False)

    # Host feeds these already transposed: IT/XT are [E, S], maskT is [S, S]
    # with [k, q] indexing. x rides HWDGE as float32r (runs concurrently with
    # the SWDGE stream, so the k/v path starts ~20us earlier and the SWDGE
    # stream is 6MB shorter); I rides the SWDGE cast path.
    F32R = mybir.dt.float32r
    dIT = nc.dram_tensor("IT", [E, S], F32, kind="ExternalInput")
    dXT = nc.dram_tensor("XT", [E, S], F32, kind="ExternalInput")
    dmT = nc.dram_tensor("maskT", [S, S], I32, kind="ExternalInput")
    # weights host-packed into one contiguous [E, 3H] bf16 tensor and biases
    # into one [H, 2H] f32 tensor (bq/bk broadcast along the row so each
    # partition moves one contiguous run; only col 0 / col H is read) — a
    # single clean DMA each instead of many fragmented small ones
    dW = nc.dram_tensor("Wpack", [E, 3 * H], BF16, kind="ExternalInput")
    dB = nc.dram_tensor("bpack", [H, 2 * H], F32, kind="ExternalInput")
    dbv = nc.dram_tensor("bv", [1, H], BF16, kind="ExternalInput")
    dout = nc.dram_tensor("out", [S, H], F32, kind="ExternalOutput")

    with tile.TileContext(nc) as tc, ExitStack() as ctx:
        singles = ctx.enter_context(tc.tile_pool(name="singles", bufs=1))

        # big input loads go first on each DGE queue
        IT = singles.tile([128, EC, S], BF16, tag="IT")
        XT = singles.tile([128, EC, S], BF16, tag="XT")
        for lo, hi in ((0, S // 2), (S // 2, S)):
            nc.gpsimd.dma_start(
                out=IT[:, :, lo:hi],
                in_=dIT.ap()[:, lo:hi].rearrange("(ec p) s -> p ec s", p=128),
            )
            nc.gpsimd.dma_start(
                out=XT[:, :, lo:hi],
                in_=dXT.ap()[:, lo:hi].rearrange("(ec p) s -> p ec s", p=128),
            )

        ones_row = singles.tile([1, S], BF16, tag="ones")
        nc.vector.memset(ones_row, 1.0)

        w_all = singles.tile([128, EC, 3 * H], BF16, tag="Wpack")
        nc.sync.dma_start(
            out=w_all, in_=dW.ap().rearrange("(ec p) h -> p ec h", p=128)
        )
        w_sb = {
            "Wq": w_all[:, :, 0:H],
            "Wk": w_all[:, :, H:2 * H],
            "Wv": w_all[:, :, 2 * H:3 * H],
        }
        b_all = singles.tile([H, 2 * H], F32, tag="bpack")
        nc.sync.dma_start(out=b_all, in_=dB.ap())
        bv_t = singles.tile([1, H], BF16, tag="bv")
        nc.sync.dma_start(out=bv_t, in_=dbv.ap())
        b_sb = {"bq": b_all[:, 0:1], "bk": b_all[:, H:H + 1], "bv": bv_t}

        # ---- projections ----
        qT = singles.tile([64, S], BF16, tag="qT")
        kT = singles.tile([64, S], BF16, tag="kT")
        vA = singles.tile([128, SC, 66], BF16, tag="vA")

        # whole mask^T resident in SBUF (64KB/partition), filled by 2MB-read
        # SWDGE cast-DMAs issued back-to-back so the SDMA stream never gates
        # on consumer pool slots (SWDGE-cast sustains ~330GB/s; HWDGE bulk
        # measured slower, and the combined rate is HBM-capped anyway)
        maskT_all = singles.tile([128, SC, S], BF16, tag="maskT")
        for ki in range(0, SC, 2):
            nc.gpsimd.dma_start(
                out=maskT_all[:, ki:ki + 2, :],
                in_=dmT.ap()[ki * 128:(ki + 2) * 128, :].rearrange(
                    "(t p) q -> p t q", p=128
                ),
            )

        psw = ctx.enter_context(tc.tile_pool(name="psw", bufs=2, space="PSUM"))
        ep = ctx.enter_context(tc.tile_pool(name="ep", bufs=4))
        outp = ctx.enter_context(tc.tile_pool(name="outp", bufs=3))

        def emit_score(ki):
            """w^T-chunk -> exp -> mask multiply; returns the sT tile."""
            maskT_sb = maskT_all[:, ki, :]
            sT_sb = ep.tile([128, S], BF16, tag="sT")
            for hh in range(2):
                wp = psw.tile([128, 1024], F32, tag="w")
                for nb in range(2):
                    nc.tensor.matmul(
                        wp[:, nb * 512:(nb + 1) * 512],
                        lhsT=kT[:, ki * 128:(ki + 1) * 128],
                        rhs=qT[:, (hh * 2 + nb) * 512:(hh * 2 + nb + 1) * 512],
                        start=True,
                        stop=True,
                    )
                e_sb = ep.tile([128, 1024], BF16, tag="e")
                nc.scalar.activation(e_sb, wp, AF.Exp, scale=SCALE)
                nc.vector.tensor_tensor(
                    sT_sb[:, hh * 1024:(hh + 1) * 1024],
                    e_sb,
                    maskT_sb[:, hh * 1024:(hh + 1) * 1024],
                    ALU.mult,
                )
            return sT_sb

        with tc.tile_pool(name="ps2", bufs=2, space="PSUM") as ps2:
            for srcT, wname, bname, dstT in (
                (IT, "Wq", "bq", qT),
                (XT, "Wk", "bk", kT),
            ):
                for nb in range(4):
                    ps = ps2.tile([64, 512], F32, tag="pqk")
                    for ei in range(EC):
                        nc.tensor.matmul(
                            ps,
                            lhsT=w_sb[wname][:, ei, :],
                            rhs=srcT[:, ei, nb * 512:(nb + 1) * 512],
                            start=(ei == 0),
                            stop=(ei == EC - 1),
                        )
                    # bias folded into the PSUM->SBUF copy on DVE
                    # (per-partition scalar add keeps ACT free for the exps)
                    nc.vector.tensor_scalar(
                        dstT[:, nb * 512:(nb + 1) * 512],
                        ps,
                        b_sb[bname],
                        None,
                        ALU.add,
                    )

            # ki=0 scores go first so the exp/mask chain starts as early as
            # possible; the v projections then fill PE slack behind it.
            sT0 = emit_score(0)

            for kb in range(SC):
                psv = ps2.tile([128, H], F32, tag="pv")
                for ei in range(EC):
                    nc.tensor.matmul(
                        psv,
                        lhsT=XT[:, ei, kb * 128:(kb + 1) * 128],
                        rhs=w_sb["Wv"][:, ei, :],
                        start=(ei == 0),
                        stop=False,
                    )
                nc.tensor.matmul(
                    psv,
                    lhsT=ones_row[:, 0:128],
                    rhs=b_sb["bv"],
                    start=False,
                    stop=True,
                )
                nc.vector.tensor_copy(vA[:, kb, 0:H], psv)
                nc.vector.memset(vA[:, kb, H:H + 1], 1.0)

        # ---- attention, k-chunk-major ----
        psctx = ctx.enter_context(tc.tile_pool(name="psctx", bufs=1, space="PSUM"))

        # [q_within, qj, 64 ctx + 1 denom + pad] — 128-wide regions keep each
        # accumulation group inside one PSUM bank.
        ctxall = psctx.tile([128, SC, 128], F32, tag="ctxall")

        def emit_ctx(ki, sT_sb):
            # start=True zeroes the whole 2KB PSUM bank, so only the first
            # matmul touching each bank (4 qj regions per bank) gets it; the
            # other ki=0 writes land on zeroed-has_written elements and
            # overwrite. stop on the bank's last matmul.
            for qj in range(SC):
                nc.tensor.matmul(
                    ctxall[:, qj, 0:H + 1],
                    lhsT=sT_sb[:, qj * 128:(qj + 1) * 128],
                    rhs=vA[:, ki, 0:H + 1],
                    start=(ki == 0 and qj % 4 == 0),
                    stop=(ki == SC - 1 and qj % 4 == 3),
                )

        # Software-pipelined by one k-chunk: ctx matmuls for ki-1 are emitted
        # after the w matmuls of ki, so the PE never waits on the exp/mask
        # chain of the chunk it just computed (keeps the PE dense and the HAM
        # clock gate warm).
        prev = (0, sT0)
        for ki in range(1, SC):
            sT_sb = emit_score(ki)
            emit_ctx(prev[0], prev[1])
            prev = (ki, sT_sb)
        emit_ctx(prev[0], prev[1])

        # vectorized epilogue: one reciprocal over all 16 denominators, one
        # free-dim-broadcast multiply, one 512KB output DMA
        recip_t = outp.tile([128, SC, 1], F32, tag="recip")
        nc.vector.reciprocal(recip_t, ctxall[:, :, H:H + 1])
        recip_bcast = bass.AP(
            tensor=recip_t.tensor,
            offset=recip_t.offset,
            ap=[recip_t.ap[0], recip_t.ap[1], [0, H]],
        )
        o_all = outp.tile([128, SC, H], F32, tag="o")
        nc.vector.tensor_tensor(o_all, ctxall[:, :, 0:H], recip_bcast, ALU.mult)
        nc.sync.dma_start(
            out=dout.ap().rearrange("(qj p) h -> p qj h", p=128), in_=o_all
        )

    nc.compile()
    return nc


def get_program():
    if "nc" not in _cache:
        _cache["nc"] = _build_program()
    return _cache["nc"]


def make_in_maps(I, x, mask, Wq, bq, Wk, bk, Wv, bv):
    I = np.asarray(I, dtype=np.float32)
    x = np.asarray(x, dtype=np.float32)
    mask = np.asarray(mask, dtype=np.int32)
    import ml_dtypes

    BF = ml_dtypes.bfloat16
    Wpack = np.concatenate(
        [
            np.asarray(Wq, dtype=np.float32).astype(BF),
            np.asarray(Wk, dtype=np.float32).astype(BF),
            np.asarray(Wv, dtype=np.float32).astype(BF),
        ],
        axis=1,
    )
    bpack = np.concatenate(
        [
            np.broadcast_to(np.asarray(bq, np.float32).reshape(H, 1), (H, H)),
            np.broadcast_to(np.asarray(bk, np.float32).reshape(H, 1), (H, H)),
        ],
        axis=1,
    ).astype(np.float32)
    bv = np.asarray(bv, dtype=np.float32).reshape(1, H).astype(BF)

    return [
        {
            "IT": np.ascontiguousarray(I[b].T),
            "XT": np.ascontiguousarray(x[b].T),
            "maskT": np.ascontiguousarray(mask[b].T),
            "Wpack": Wpack, "bpack": bpack, "bv": bv,
        }
        for b in range(B)
    ]


def kernel(I, x, mask, Wq, bq, Wk, bk, Wv, bv):
    nc = get_program()
    in_maps = make_in_maps(I, x, mask, Wq, bq, Wk, bk, Wv, bv)
    res = run_bass_kernel_spmd(nc, in_maps, list(range(N_CORES)))
    out = np.stack([res.results[b]["out"] for b in range(B)], axis=0)
    return out.astype(np.float32)
